# revision 1
# baseline (speedup 1.0000x reference)
"""Multi-head attention Trainium2 Bass kernel, v2 (bf16).

Problem: B=4, T=2048, D=1024, H=16 heads of dim 64 (fp32 interface).
  qkv = x @ Wqkv.T + bqkv ; per-head attention ; out @ Wo.T + bo

Sharding: 8 cores = 4 batches x 2 head-groups of 8 heads.  Each core
computes its batch's attention for its 8 heads plus the out-projection
restricted to its heads' columns (partial sum); the host adds the two
partial outputs per batch and transposes back.

v2 changes vs v1:
  - all matmul operands in bf16 (PSUM accum stays fp32); SBUF footprint
    halves so every head-pack's Q^T/K^T lives in SBUF at once.
  - phase-separated schedule: (1) V+QK generation for all packs,
    (2) attention (ACT-engine/exp-bound), (3) out-projection.  PSUM
    banks partition cleanly per phase (gen: 8x512-wide, attention:
    2x1024 score tiles + 2x1024 out tiles = 8 banks).
  - S matmuls ordered A-major (A sh0, A sh1, B sh0, B sh1) so exp(A)
    unblocks after 2 matmuls -> ACT engine saturates earlier.
  - V-projection bias folded into the host-precomputed output bias
    (bo' = bo*[g==0] + Wo_g @ bv_g), removing the per-tile bias add
    from the attention epilogue.
  - x/weight DMA in token-block-major order so generation starts after
    ~1/4 of the x transfer.
"""

import sys

sys.path.insert(0, "/opt/trn_rl_repo")

import numpy as np

import concourse.bass as bass  # noqa: F401
from concourse import bacc
import concourse.mybir as mybir
import concourse.tile as tile
from concourse.bass_utils import run_bass_kernel_spmd

B, T, D = 4, 2048, 1024
H, HD = 16, 64
P = 128
FP32 = mybir.dt.float32
BF16 = mybir.dt.bfloat16
AF = mybir.ActivationFunctionType
OP = mybir.AluOpType

N_CORES = 8
HPC = 8          # heads per core
NPACK = 4        # head pairs per core
CT = D // P      # 8 contraction tiles over D
KT = T // P      # 16 key tiles
QH = 2           # q halves
QHW = T // QH    # 1024
TBW = 512        # token block width for gen/out-proj
NTB = T // TBW   # 4
SCALE = HD ** -0.5


def build_nc(reps: int = 1, variant: str = "base", dyn: bool = False):
    nc = bacc.Bacc(None, target_bir_lowering=False, debug=False)

    xT_d = nc.dram_tensor("xT", [D, T], BF16, kind="ExternalInput")
    wqkT_d = nc.dram_tensor("wqkT", [D, NPACK, 256], BF16, kind="ExternalInput")
    bqk_d = nc.dram_tensor("bqk", [P, NPACK, 2], FP32, kind="ExternalInput")
    wvT_d = nc.dram_tensor("wvT", [D, HPC * HD], BF16, kind="ExternalInput")
    woT_d = nc.dram_tensor("woT", [NPACK * P, D], BF16, kind="ExternalInput")
    bo_d = nc.dram_tensor("bo", [P, CT], FP32, kind="ExternalInput")
    if dyn:
        nreps_d = nc.dram_tensor("nreps", [1, 1], mybir.dt.int32,
                                 kind="ExternalInput")
    yT_d = nc.dram_tensor("yT", [D, T], FP32, kind="ExternalOutput")

    with tile.TileContext(nc) as tc:
        with (
            tc.tile_pool(name="persist", bufs=1) as persist,
        ):
            ones_col = nc.const_aps.tensor(1.0, [P, 1], FP32)

            # ---- persistent SBUF residents (bf16) -------------------------
            xts = persist.tile([P, CT, T], BF16, tag="xts")           # 32 KB/p
            qkts = persist.tile([P, NPACK, 2, T], BF16, tag="qkts")   # 32 KB/p
            vps = persist.tile([P, KT, HPC * (HD + 1)], BF16, tag="vps")  # 16.25
            attnT = persist.tile([P, NPACK, T], BF16, tag="attnT")    # 16 KB/p
            woTs = persist.tile([P, NPACK, D], BF16, tag="woTs")      # 8 KB/p
            bqks = persist.tile([P, NPACK, 2], FP32, tag="bqks")
            bos = persist.tile([P, CT], FP32, tag="bos")

            # token-block-major x DMA so gen can start early
            for ttg in range(NTB):
                for ct in range(CT):
                    nc.sync.dma_start(
                        xts[:, ct, ttg * TBW:(ttg + 1) * TBW],
                        xT_d[ct * P:(ct + 1) * P, ttg * TBW:(ttg + 1) * TBW])
            nc.sync.dma_start(bqks[:], bqk_d[:, :, :])
            nc.sync.dma_start(bos[:], bo_d[:, :])
            for ci in range(NPACK):
                nc.sync.dma_start(woTs[:, ci, :], woT_d[ci * P:(ci + 1) * P, :])

            att_variants = ("attonly", "attpad", "sonly", "seonly", "pvonly",
                            "expchain", "pvsingle", "pvchunk")
            if variant in att_variants:
                nc.vector.memset(vps[:].bitcast(mybir.dt.uint16), 0)
                nc.vector.memset(qkts[:].bitcast(mybir.dt.uint16), 0)
            if dyn:
                nrt_sb = persist.tile([1, 1], mybir.dt.int32, tag="nrt")
                nc.sync.dma_start(nrt_sb[:], nreps_d[:, :])
                nval = nc.values_load(nrt_sb[0:1, 0:1], min_val=1,
                                      max_val=1 << 20,
                                      skip_runtime_bounds_check=True)
                rep_ctx = tc.For_i(0, nval, 1)
            else:
                rep_ctx = None

            import contextlib
            with rep_ctx if rep_ctx is not None else contextlib.nullcontext():
              for _ in range(reps):
                # ---- phase 1: V + QK generation (all packs) --------------
                if variant not in att_variants:
                  with (
                      tc.tile_pool(name="wv_pool", bufs=1) as wv_pool,
                      tc.tile_pool(name="wqk_pool", bufs=2) as wqk_pool,
                      tc.tile_pool(name="genpsum", bufs=4,
                                   space="PSUM") as genpsum,
                  ):
                      wvs = wv_pool.tile([P, CT, HPC * HD], BF16, tag="wvs")
                      for ct in range(CT):
                          nc.sync.dma_start(
                              wvs[:, ct, :], wvT_d[ct * P:(ct + 1) * P, :])
                      for tt in range(KT):
                          vview = vps[:, tt, :].rearrange(
                              "p (h e) -> p h e", h=HPC)
                          nc.vector.tensor_copy(
                              vview[:, :, HD:HD + 1],
                              ones_col.to_broadcast([P, HPC, 1]))
                      for tt in range(KT):
                          ps = genpsum.tile([P, TBW], FP32, tag="gp")
                          for ct in range(CT):
                              nc.tensor.matmul(
                                  ps[:],
                                  xts[:, ct, tt * P:(tt + 1) * P],
                                  wvs[:, ct, :],
                                  start=(ct == 0), stop=(ct == CT - 1))
                          vview = vps[:, tt, :].rearrange(
                              "p (h e) -> p h e", h=HPC)
                          nc.vector.tensor_copy(
                              vview[:, :, 0:HD],
                              ps.rearrange("p (h d) -> p h d", h=HPC))

                      for p in range(NPACK):
                          wqk = wqk_pool.tile([P, CT, 256], BF16, tag="wqk")
                          for ct in range(CT):
                              nc.sync.dma_start(
                                  wqk[:, ct, :],
                                  wqkT_d[ct * P:(ct + 1) * P, p, :])
                          for jj in range(2):
                              for tb in range(NTB):
                                  ps = genpsum.tile([P, TBW], FP32, tag="gp")
                                  for ct in range(CT):
                                      nc.tensor.matmul(
                                          ps[:],
                                          wqk[:, ct, jj * P:(jj + 1) * P],
                                          xts[:, ct,
                                              tb * TBW:(tb + 1) * TBW],
                                          start=(ct == 0),
                                          stop=(ct == CT - 1))
                                  nc.vector.tensor_scalar_add(
                                      qkts[:, p, jj, tb * TBW:(tb + 1) * TBW],
                                      ps[:], bqks[:, p, jj:jj + 1])

                # ---- phase 2: attention ----------------------------------
                if variant != "genonly":
                  with (
                      tc.tile_pool(name="pt_pool", bufs=6) as pt_pool,
                      tc.tile_pool(name="rep_pool", bufs=2) as rep_pool,
                      tc.tile_pool(name="spsum", bufs=2,
                                   space="PSUM") as spsum,
                      tc.tile_pool(name="outpsum", bufs=2,
                                   space="PSUM") as outpsum,
                  ):
                    if variant == "pvonly":
                        pt_fix = [pt_pool.tile([P, QHW], BF16, tag="ptf",
                                               name=f"ptf{i}")
                                  for i in range(2)]
                        for t in pt_fix:
                            nc.vector.memset(t[:].bitcast(mybir.dt.uint16), 0)
                    if variant == "expchain":
                        sps_fix = spsum.tile([P, QHW], FP32, tag="spsf")
                        nc.tensor.matmul(
                            sps_fix[:, 0:512],
                            qkts[0:HD, 0, 1, 0:P], qkts[0:HD, 0, 0, 0:512],
                            start=True, stop=True)
                        nc.tensor.matmul(
                            sps_fix[:, 512:1024],
                            qkts[0:HD, 0, 1, 0:P], qkts[0:HD, 0, 0, 0:512],
                            start=True, stop=True)
                    for p in range(NPACK):
                      for qh in range(QH):
                        q0 = qh * QHW
                        do_pv = variant in ("base", "basepad", "attonly", "attpad", "pvonly",
                                            "pvsingle", "pvchunk")
                        do_s = variant in ("base", "basepad", "attonly", "attpad",
                                           "sonly", "seonly", "pvsingle", "pvchunk")
                        do_exp = variant in ("base", "basepad", "attonly", "attpad",
                                             "seonly", "expchain", "pvsingle",
                                             "pvchunk")
                        do_epi = variant in ("base", "basepad", "attonly", "attpad")
                        if do_pv and variant != "pvchunk":
                            outA = outpsum.tile([P, QHW], FP32, tag="outp")
                            outB = outpsum.tile([P, QHW], FP32, tag="outp")
                        else:
                            outA = outB = None
                        chunk_tiles = {}
                        halves = [(0, HD, outA, 2 * p), (HD, P, outB, 2 * p + 1)]
                        prev = None

                        CHUNK = 4
                        if variant == "pvchunk":
                            acc = [rep_pool.tile([P, QHW], FP32, tag="acc",
                                                 name=f"acc{i}")
                                   for i in range(2)]

                        def emit_pv(entry):
                            ktp, pts = entry
                            if variant == "pvsingle":
                                st, sp = True, True
                            elif variant == "pvchunk":
                                st = (ktp % CHUNK == 0)
                                sp = (ktp % CHUNK == CHUNK - 1)
                            else:
                                st = (ktp == 0)
                                sp = (ktp == KT - 1)
                            for hi_, ((lo, hi, outp, hloc), pt) in enumerate(
                                    zip(halves, pts)):
                                if variant == "pvchunk":
                                    if st:
                                        chunk_tiles[hi_] = outpsum.tile(
                                            [P, QHW], FP32, tag="outp",
                                            name=f"oc{hloc}{ktp}")
                                    outp = chunk_tiles[hi_]
                                for sh in range(QHW // 512):
                                    nc.tensor.matmul(
                                        outp[0:HD + 1, sh * 512:(sh + 1) * 512],
                                        vps[:, ktp,
                                            hloc * (HD + 1):(hloc + 1) * (HD + 1)],
                                        pt[:, sh * 512:(sh + 1) * 512],
                                        start=st, stop=sp)
                                if variant == "pvchunk" and sp:
                                    if ktp < CHUNK:
                                        nc.vector.tensor_copy(
                                            acc[hi_][0:HD + 1, :],
                                            outp[0:HD + 1, :])
                                    else:
                                        nc.vector.tensor_tensor(
                                            acc[hi_][0:HD + 1, :],
                                            acc[hi_][0:HD + 1, :],
                                            outp[0:HD + 1, :], OP.add)

                        pad = 2 if variant in ("attpad", "basepad") else 0
                        for kt in range(KT):
                            pts = []
                            # A-major: both sh chunks of a half, then its exp
                            for (lo, hi, outp, hloc) in halves:
                                if do_s:
                                    sps = spsum.tile([P, QHW], FP32, tag="sps")
                                    for _ in range(pad):
                                        nc.tensor.matmul(
                                            sps[:, 0:512],
                                            qkts[lo:hi, p, 1, 0:P],
                                            qkts[lo:hi, p, 0, 0:512],
                                            start=True, stop=True)
                                    for sh in range(QHW // 512):
                                        nc.tensor.matmul(
                                            sps[:, sh * 512:(sh + 1) * 512],
                                            qkts[lo:hi, p, 1,
                                                 kt * P:(kt + 1) * P],
                                            qkts[lo:hi, p, 0,
                                                 q0 + sh * 512:
                                                 q0 + (sh + 1) * 512],
                                            start=True, stop=True)
                                elif variant == "expchain":
                                    sps = sps_fix
                                if do_exp:
                                    pt = pt_pool.tile([P, QHW], BF16, tag="pt")
                                    nc.scalar.activation(
                                        pt[:], sps[:], AF.Exp, scale=SCALE)
                                    pts.append(pt)
                                elif variant == "pvonly":
                                    pts.append(pt_fix[hloc % 2])
                            if do_pv:
                                if prev is not None:
                                    emit_pv(prev)
                                prev = (kt, pts)
                        if do_pv:
                            emit_pv(prev)

                        # normalize into attnT (A rows 0:64, B rows 64:128)
                        if do_epi:
                          for row0, outp in [(0, outA), (HD, outB)]:
                            rep = rep_pool.tile([HD, QHW], FP32, tag="rep")
                            nc.vector.reciprocal(
                                rep[0:1, :], outp[HD:HD + 1, :])
                            nc.gpsimd.partition_broadcast(
                                rep[:], rep[0:1, :])
                            nc.vector.tensor_tensor(
                                attnT[row0:row0 + HD, p, q0:q0 + QHW],
                                outp[0:HD, :], rep[:], OP.mult)

                # ---- phase 3: out projection -----------------------------
                if variant not in att_variants:
                  with (
                      tc.tile_pool(name="opsum", bufs=4,
                                   space="PSUM") as opsum,
                      tc.tile_pool(name="ystage_pool", bufs=4) as ystage_pool,
                  ):
                    for co in range(CT):
                      for tb in range(NTB):
                        ps = opsum.tile([P, TBW], FP32, tag="op")
                        for ci in range(NPACK):
                            nc.tensor.matmul(
                                ps[:],
                                woTs[:, ci, co * P:(co + 1) * P],
                                attnT[:, ci, tb * TBW:(tb + 1) * TBW],
                                start=(ci == 0), stop=(ci == NPACK - 1))
                        yst = ystage_pool.tile([P, TBW], FP32, tag="yst")
                        nc.vector.tensor_scalar_add(
                            yst[:], ps[:], bos[:, co:co + 1])
                        nc.sync.dma_start(
                            yT_d[co * P:(co + 1) * P, tb * TBW:(tb + 1) * TBW],
                            yst[:])
    nc.compile()
    return nc


def _to_bf16(a):
    import ml_dtypes
    return np.asarray(a, np.float32).astype(ml_dtypes.bfloat16)


def _prep_core_inputs(x, Wqkv, bqkv, Wo, bo, core):
    b, g = core // 2, core % 2
    f32 = np.float32

    xT = _to_bf16(np.ascontiguousarray(x[b].T))

    wqkT = np.empty((D, NPACK, 256), f32)
    bqk = np.empty((P, NPACK, 2), f32)
    for p in range(NPACK):
        rows_q, rows_k = [], []
        for j in range(2):
            h = 8 * g + 2 * p + j
            rows_q.append(slice(192 * h, 192 * h + 64))
            rows_k.append(slice(192 * h + 64, 192 * h + 128))
        Q2 = np.vstack([Wqkv[rows_q[0]], Wqkv[rows_q[1]]])   # [128, D]
        K2 = np.vstack([Wqkv[rows_k[0]], Wqkv[rows_k[1]]])
        wqkT[:, p, :128] = Q2.T
        wqkT[:, p, 128:] = K2.T
        bqk[:, p, 0] = np.concatenate([bqkv[rows_q[0]], bqkv[rows_q[1]]])
        bqk[:, p, 1] = np.concatenate([bqkv[rows_k[0]], bqkv[rows_k[1]]])

    rows_v = [slice(192 * (8 * g + h) + 128, 192 * (8 * g + h) + 192)
              for h in range(HPC)]
    Wv = np.vstack([Wqkv[r] for r in rows_v])                # [512, D]
    wvT = _to_bf16(np.ascontiguousarray(Wv.T))

    woT = np.ascontiguousarray(Wo[:, 512 * g:512 * (g + 1)].T)  # [512, D]

    # fold V-bias through the out-projection: bo' = [g==0]*bo + woT.T @ bv
    bv_flat = np.empty(512, f32)
    for p in range(NPACK):
        bv_flat[128 * p:128 * p + 64] = bqkv[rows_v[2 * p]]
        bv_flat[128 * p + 64:128 * (p + 1)] = bqkv[rows_v[2 * p + 1]]
    bo_eff = (bo.astype(f32) if g == 0 else np.zeros(D, f32)) \
        + woT.T.astype(f32) @ bv_flat
    bo2 = np.ascontiguousarray(bo_eff.reshape(CT, P).T)

    return {
        "xT": xT, "wqkT": _to_bf16(wqkT), "bqk": bqk, "wvT": wvT,
        "woT": _to_bf16(woT), "bo": bo2,
    }


_NC_CACHE = {}


def kernel(x, Wqkv, bqkv, Wo, bo, _reps: int = 1,
           _return_raw: bool = False):
    x = np.asarray(x, np.float32)
    Wqkv = np.asarray(Wqkv, np.float32)
    bqkv = np.asarray(bqkv, np.float32)
    Wo = np.asarray(Wo, np.float32)
    bo = np.asarray(bo, np.float32)

    in_maps = [_prep_core_inputs(x, Wqkv, bqkv, Wo, bo, c)
               for c in range(N_CORES)]

    if _reps not in _NC_CACHE:
        _NC_CACHE[_reps] = build_nc(_reps)
    nc = _NC_CACHE[_reps]

    res = run_bass_kernel_spmd(nc, in_maps, core_ids=list(range(N_CORES)))
    if _return_raw:
        return res

    y = np.empty((B, T, D), np.float32)
    for b in range(B):
        yt = res.results[2 * b]["yT"] + res.results[2 * b + 1]["yT"]
        y[b] = yt.T
    return y



# revision 14
# speedup vs baseline: 1.0003x; 1.0003x over previous
"""Multi-head attention Trainium2 Bass kernel, v2 (bf16).

Problem: B=4, T=2048, D=1024, H=16 heads of dim 64 (fp32 interface).
  qkv = x @ Wqkv.T + bqkv ; per-head attention ; out @ Wo.T + bo

Sharding: 8 cores = 4 batches x 2 head-groups of 8 heads.  Each core
computes its batch's attention for its 8 heads plus the out-projection
restricted to its heads' columns (partial sum); the host adds the two
partial outputs per batch and transposes back.

v2 changes vs v1:
  - all matmul operands in bf16 (PSUM accum stays fp32); SBUF footprint
    halves so every head-pack's Q^T/K^T lives in SBUF at once.
  - phase-separated schedule: (1) V+QK generation for all packs,
    (2) attention (ACT-engine/exp-bound), (3) out-projection.  PSUM
    banks partition cleanly per phase (gen: 8x512-wide, attention:
    2x1024 score tiles + 2x1024 out tiles = 8 banks).
  - S matmuls ordered A-major (A sh0, A sh1, B sh0, B sh1) so exp(A)
    unblocks after 2 matmuls -> ACT engine saturates earlier.
  - V-projection bias folded into the host-precomputed output bias
    (bo' = bo*[g==0] + Wo_g @ bv_g), removing the per-tile bias add
    from the attention epilogue.
  - x/weight DMA in token-block-major order so generation starts after
    ~1/4 of the x transfer.
"""

import sys

sys.path.insert(0, "/opt/trn_rl_repo")

import numpy as np

import concourse.bass as bass  # noqa: F401
from concourse import bacc
import concourse.mybir as mybir
import concourse.tile as tile
from concourse.bass_utils import run_bass_kernel_spmd

B, T, D = 4, 2048, 1024
H, HD = 16, 64
P = 128
FP32 = mybir.dt.float32
BF16 = mybir.dt.bfloat16
AF = mybir.ActivationFunctionType
OP = mybir.AluOpType

N_CORES = 8
HPC = 8          # heads per core
NPACK = 4        # head pairs per core
CT = D // P      # 8 contraction tiles over D
KT = T // P      # 16 key tiles
QH = 2           # q halves
QHW = T // QH    # 1024
TBW = 512        # token block width for gen/out-proj
NTB = T // TBW   # 4
SCALE = HD ** -0.5

# fast-exp (Schraudolph, bf16 bit pattern): i16 = s*(2^7*log2e*SCALE) + B
# B = 127*2^7 - 5.5 centers the sawtooth error (trunc-convert model).
AEXP = float(np.float32(2.0 ** 7 / np.log(2.0) * 0.125))
BEXP = float(np.float32(16250.5))
DVE_KTS = frozenset({1, 3, 5, 7, 9, 11, 13})   # 7 of 16 kts exp'd on DVE


def build_nc(reps: int = 1, variant: str = "base", dyn: bool = False):
    nc = bacc.Bacc(None, target_bir_lowering=False, debug=False)

    xT_d = nc.dram_tensor("xT", [D, T], BF16, kind="ExternalInput")
    wqkT_d = nc.dram_tensor("wqkT", [D, NPACK, 256], BF16, kind="ExternalInput")
    bqk_d = nc.dram_tensor("bqk", [P, NPACK, 2], FP32, kind="ExternalInput")
    wvT_d = nc.dram_tensor("wvT", [D, HPC * HD], BF16, kind="ExternalInput")
    woT_d = nc.dram_tensor("woT", [NPACK * P, D], BF16, kind="ExternalInput")
    bo_d = nc.dram_tensor("bo", [P, CT], FP32, kind="ExternalInput")
    if dyn:
        nreps_d = nc.dram_tensor("nreps", [1, 1], mybir.dt.int32,
                                 kind="ExternalInput")
    yT_d = nc.dram_tensor("yT", [D, T], FP32, kind="ExternalOutput")

    with tile.TileContext(nc) as tc:
        with (
            tc.tile_pool(name="persist", bufs=1) as persist,
        ):
            ones_col = nc.const_aps.tensor(1.0, [P, 1], FP32)

            # ---- persistent SBUF residents (bf16) -------------------------
            xts = persist.tile([P, CT, T], BF16, tag="xts")           # 32 KB/p
            qkts = persist.tile([P, NPACK, 2, T], BF16, tag="qkts")   # 32 KB/p
            vps = persist.tile([P, KT, HPC * (HD + 1)], BF16, tag="vps")  # 16.25
            attnT = persist.tile([P, NPACK, T], BF16, tag="attnT")    # 16 KB/p
            woTs = persist.tile([P, NPACK, D], BF16, tag="woTs")      # 8 KB/p
            bqks = persist.tile([P, NPACK, 2], FP32, tag="bqks")
            bos = persist.tile([P, CT], FP32, tag="bos")

            # token-block-major x DMA so gen can start early
            for ttg in range(NTB):
                for ct in range(CT):
                    nc.sync.dma_start(
                        xts[:, ct, ttg * TBW:(ttg + 1) * TBW],
                        xT_d[ct * P:(ct + 1) * P, ttg * TBW:(ttg + 1) * TBW])
            nc.sync.dma_start(bqks[:], bqk_d[:, :, :])
            nc.sync.dma_start(bos[:], bo_d[:, :])
            for ci in range(NPACK):
                nc.sync.dma_start(woTs[:, ci, :], woT_d[ci * P:(ci + 1) * P, :])

            att_variants = ("attonly", "attpad", "sonly", "seonly", "pvonly",
                            "expchain", "pvsingle", "pvchunk",
                            "pvm128", "pvfp8", "sexp_dve", "expchain_dve")
            if variant in att_variants:
                nc.vector.memset(vps[:].bitcast(mybir.dt.uint16), 0)
                nc.vector.memset(qkts[:].bitcast(mybir.dt.uint16), 0)
            if variant == "genonly":
                nc.vector.memset(attnT[:].bitcast(mybir.dt.uint16), 0)
            if dyn:
                nrt_sb = persist.tile([1, 1], mybir.dt.int32, tag="nrt")
                nc.sync.dma_start(nrt_sb[:], nreps_d[:, :])
                nval = nc.values_load(nrt_sb[0:1, 0:1], min_val=1,
                                      max_val=1 << 20,
                                      skip_runtime_bounds_check=True)
                rep_ctx = tc.For_i(0, nval, 1)
            else:
                rep_ctx = None

            import contextlib
            with rep_ctx if rep_ctx is not None else contextlib.nullcontext():
              for _ in range(reps):
                # ---- phase 1: V + QK generation (all packs) --------------
                if variant not in att_variants:
                  with (
                      tc.tile_pool(name="wv_pool", bufs=1) as wv_pool,
                      tc.tile_pool(name="wqk_pool", bufs=2) as wqk_pool,
                      tc.tile_pool(name="genpsum", bufs=4,
                                   space="PSUM") as genpsum,
                  ):
                      wvs = wv_pool.tile([P, CT, HPC * HD], BF16, tag="wvs")
                      for ct in range(CT):
                          nc.sync.dma_start(
                              wvs[:, ct, :], wvT_d[ct * P:(ct + 1) * P, :])
                      for tt in range(KT):
                          vview = vps[:, tt, :].rearrange(
                              "p (h e) -> p h e", h=HPC)
                          nc.vector.tensor_copy(
                              vview[:, :, HD:HD + 1],
                              ones_col.to_broadcast([P, HPC, 1]))
                      for tt in range(KT):
                          ps = genpsum.tile([P, TBW], FP32, tag="gp")
                          for ct in range(CT):
                              nc.tensor.matmul(
                                  ps[:],
                                  xts[:, ct, tt * P:(tt + 1) * P],
                                  wvs[:, ct, :],
                                  start=(ct == 0), stop=(ct == CT - 1))
                          vview = vps[:, tt, :].rearrange(
                              "p (h e) -> p h e", h=HPC)
                          # alternate ACT/DVE so both engines drain gen psum
                          if tt % 2 == 0:
                              nc.scalar.copy(
                                  vview[:, :, 0:HD],
                                  ps.rearrange("p (h d) -> p h d", h=HPC))
                          else:
                              nc.vector.tensor_copy(
                                  vview[:, :, 0:HD],
                                  ps.rearrange("p (h d) -> p h d", h=HPC))

                      for p in range(NPACK):
                          wqk = wqk_pool.tile([P, CT, 256], BF16, tag="wqk")
                          for ct in range(CT):
                              nc.sync.dma_start(
                                  wqk[:, ct, :],
                                  wqkT_d[ct * P:(ct + 1) * P, p, :])
                          for jj in range(2):
                              for tb in range(NTB):
                                  ps = genpsum.tile([P, TBW], FP32, tag="gp")
                                  for ct in range(CT):
                                      nc.tensor.matmul(
                                          ps[:],
                                          wqk[:, ct, jj * P:(jj + 1) * P],
                                          xts[:, ct,
                                              tb * TBW:(tb + 1) * TBW],
                                          start=(ct == 0),
                                          stop=(ct == CT - 1))
                                  if tb % 2 == 0:
                                      nc.scalar.activation(
                                          qkts[:, p, jj,
                                               tb * TBW:(tb + 1) * TBW],
                                          ps[:], AF.Identity,
                                          bias=bqks[:, p, jj:jj + 1])
                                  else:
                                      nc.vector.tensor_scalar_add(
                                          qkts[:, p, jj,
                                               tb * TBW:(tb + 1) * TBW],
                                          ps[:], bqks[:, p, jj:jj + 1])

                # ---- probe variants --------------------------------------
                if variant == "pvm128":
                    # PV-like MMs with M=128 (no ones row): isolates the
                    # M=65 cost theory.  512 MMs, timing-only numerics.
                    with (
                        tc.tile_pool(name="pp", bufs=2) as pp,
                        tc.tile_pool(name="op2", bufs=2,
                                     space="PSUM") as op2,
                    ):
                        ptf = [pp.tile([P, QHW], BF16, tag="ptf",
                                       name=f"pf{i}") for i in range(2)]
                        for t in ptf:
                            nc.vector.memset(t[:].bitcast(mybir.dt.uint16), 0)
                        for rep in range(8):
                            outs = [op2.tile([P, QHW], FP32, tag="om",
                                             name=f"om{rep}_{i}")
                                    for i in range(2)]
                            for kt in range(KT):
                                for j in range(2):
                                    for sh in range(2):
                                        nc.tensor.matmul(
                                            outs[j][:, sh * 512:(sh + 1) * 512],
                                            vps[:, kt, 0:128],
                                            ptf[j][:, sh * 512:(sh + 1) * 512],
                                            start=(kt == 0), stop=(kt == KT - 1))
                if variant == "pvfp8":
                    # DoubleRow fp8 PV probe: 256 MMs, K=256 virtual.
                    FP8 = mybir.dt.float8e4
                    with (
                        tc.tile_pool(name="pp8", bufs=3) as pp8,
                        tc.tile_pool(name="op8", bufs=2,
                                     space="PSUM") as op8,
                    ):
                        v8 = pp8.tile([P, 2, 80], FP8, tag="v8")
                        pt8 = [pp8.tile([P, 2, 512], FP8, tag="p8",
                                        name=f"p8{i}") for i in range(2)]
                        nc.vector.memset(v8[:].bitcast(mybir.dt.uint8), 0)
                        for t in pt8:
                            nc.vector.memset(t[:].bitcast(mybir.dt.uint8), 0)
                        for rep in range(8):
                            outs = [op8.tile([P, QHW], FP32, tag="o8",
                                             name=f"o8{rep}_{i}")
                                    for i in range(2)]
                            for ktp in range(KT // 2):
                                for j in range(2):
                                    for sh in range(2):
                                        nc.tensor.matmul(
                                            outs[j][0:HD + 1,
                                                    sh * 512:(sh + 1) * 512],
                                            v8[:, :, 0:HD + 1],
                                            pt8[j][:, :, :],
                                            start=(ktp == 0),
                                            stop=(ktp == KT // 2 - 1),
                                            perf_mode=(
                                                mybir.MatmulPerfMode.DoubleRow))
                if variant in ("sexp_dve", "expchain_dve"):
                    # S matmuls + DVE fast-exp (Schraudolph bf16-bits):
                    # measures DVE exp rate and validates lowering.
                    with (
                        tc.tile_pool(name="ptd", bufs=6) as ptd,
                        tc.tile_pool(name="spd", bufs=2,
                                     space="PSUM") as spd,
                    ):
                        if variant == "expchain_dve":
                            spsf = spd.tile([P, QHW], FP32, tag="spf")
                            nc.tensor.matmul(
                                spsf[:, 0:512], qkts[0:HD, 0, 1, 0:P],
                                qkts[0:HD, 0, 0, 0:512], start=True, stop=True)
                            nc.tensor.matmul(
                                spsf[:, 512:1024], qkts[0:HD, 0, 1, 0:P],
                                qkts[0:HD, 0, 0, 0:512], start=True, stop=True)
                        for p in range(NPACK):
                          for qh in range(QH):
                            q0 = qh * QHW
                            for kt in range(KT):
                              for (lo, hi) in ((0, HD), (HD, P)):
                                if variant == "sexp_dve":
                                    sps = spd.tile([P, QHW], FP32, tag="spd")
                                    for sh in range(QHW // 512):
                                        nc.tensor.matmul(
                                            sps[:, sh * 512:(sh + 1) * 512],
                                            qkts[lo:hi, p, 1,
                                                 kt * P:(kt + 1) * P],
                                            qkts[lo:hi, p, 0,
                                                 q0 + sh * 512:
                                                 q0 + (sh + 1) * 512],
                                            start=True, stop=True)
                                else:
                                    sps = spsf
                                pt = ptd.tile([P, QHW], BF16, tag="ptd")
                                nc.vector.tensor_scalar(
                                    pt[:].bitcast(mybir.dt.uint16),
                                    sps[:], AEXP, BEXP, OP.mult, OP.add)

                # ---- phase 2: attention ----------------------------------
                if variant != "genonly":
                  with (
                      tc.tile_pool(name="pt_pool", bufs=6) as pt_pool,
                      tc.tile_pool(name="rep_pool", bufs=2) as rep_pool,
                      tc.tile_pool(name="spsum", bufs=2,
                                   space="PSUM") as spsum,
                      tc.tile_pool(name="outpsum", bufs=2,
                                   space="PSUM") as outpsum,
                  ):
                    if variant == "pvonly":
                        pt_fix = [pt_pool.tile([P, QHW], BF16, tag="ptf",
                                               name=f"ptf{i}")
                                  for i in range(2)]
                        for t in pt_fix:
                            nc.vector.memset(t[:].bitcast(mybir.dt.uint16), 0)
                    if variant == "expchain":
                        sps_fix = spsum.tile([P, QHW], FP32, tag="spsf")
                        nc.tensor.matmul(
                            sps_fix[:, 0:512],
                            qkts[0:HD, 0, 1, 0:P], qkts[0:HD, 0, 0, 0:512],
                            start=True, stop=True)
                        nc.tensor.matmul(
                            sps_fix[:, 512:1024],
                            qkts[0:HD, 0, 1, 0:P], qkts[0:HD, 0, 0, 0:512],
                            start=True, stop=True)
                    for p in range(NPACK):
                      for qh in range(QH):
                        q0 = qh * QHW
                        do_pv = variant in ("base", "basepad", "attonly", "attpad", "pvonly",
                                            "pvsingle", "pvchunk")
                        do_s = variant in ("base", "basepad", "attonly", "attpad",
                                           "sonly", "seonly", "pvsingle", "pvchunk")
                        do_exp = variant in ("base", "basepad", "attonly", "attpad",
                                             "seonly", "expchain", "pvsingle",
                                             "pvchunk")
                        do_epi = variant in ("base", "basepad", "attonly", "attpad")
                        if do_pv and variant != "pvchunk":
                            outA = outpsum.tile([P, QHW], FP32, tag="outp")
                            outB = outpsum.tile([P, QHW], FP32, tag="outp")
                        else:
                            outA = outB = None
                        chunk_tiles = {}
                        halves = [(0, HD, outA, 2 * p), (HD, P, outB, 2 * p + 1)]
                        prev = None

                        CHUNK = 4
                        if variant == "pvchunk":
                            acc = [rep_pool.tile([P, QHW], FP32, tag="acc",
                                                 name=f"acc{i}")
                                   for i in range(2)]

                        def emit_pv(entry):
                            ktp, pts = entry
                            if variant == "pvsingle":
                                st, sp = True, True
                            elif variant == "pvchunk":
                                st = (ktp % CHUNK == 0)
                                sp = (ktp % CHUNK == CHUNK - 1)
                            else:
                                st = (ktp == 0)
                                sp = (ktp == KT - 1)
                            for hi_, ((lo, hi, outp, hloc), pt) in enumerate(
                                    zip(halves, pts)):
                                if variant == "pvchunk":
                                    if st:
                                        chunk_tiles[hi_] = outpsum.tile(
                                            [P, QHW], FP32, tag="outp",
                                            name=f"oc{hloc}{ktp}")
                                    outp = chunk_tiles[hi_]
                                for sh in range(QHW // 512):
                                    nc.tensor.matmul(
                                        outp[0:HD + 1, sh * 512:(sh + 1) * 512],
                                        vps[:, ktp,
                                            hloc * (HD + 1):(hloc + 1) * (HD + 1)],
                                        pt[:, sh * 512:(sh + 1) * 512],
                                        start=st, stop=sp)
                                if variant == "pvchunk" and sp:
                                    if ktp < CHUNK:
                                        nc.vector.tensor_copy(
                                            acc[hi_][0:HD + 1, :],
                                            outp[0:HD + 1, :])
                                    else:
                                        nc.vector.tensor_tensor(
                                            acc[hi_][0:HD + 1, :],
                                            acc[hi_][0:HD + 1, :],
                                            outp[0:HD + 1, :], OP.add)

                        pad = 2 if variant in ("attpad", "basepad") else 0
                        for kt in range(KT):
                            pts = []
                            # A-major: both sh chunks of a half, then its exp
                            for (lo, hi, outp, hloc) in halves:
                                if do_s:
                                    sps = spsum.tile([P, QHW], FP32, tag="sps")
                                    for _ in range(pad):
                                        nc.tensor.matmul(
                                            sps[:, 0:512],
                                            qkts[lo:hi, p, 1, 0:P],
                                            qkts[lo:hi, p, 0, 0:512],
                                            start=True, stop=True)
                                    for sh in range(QHW // 512):
                                        nc.tensor.matmul(
                                            sps[:, sh * 512:(sh + 1) * 512],
                                            qkts[lo:hi, p, 1,
                                                 kt * P:(kt + 1) * P],
                                            qkts[lo:hi, p, 0,
                                                 q0 + sh * 512:
                                                 q0 + (sh + 1) * 512],
                                            start=True, stop=True)
                                elif variant == "expchain":
                                    sps = sps_fix
                                if do_exp:
                                    pt = pt_pool.tile([P, QHW], BF16, tag="pt")
                                    if kt in DVE_KTS and variant in (
                                            "base", "basepad", "attonly"):
                                        # Schraudolph fast-exp: bf16 bit
                                        # pattern via fp32 mul-add + uint16
                                        # convert on the DVE.
                                        nc.vector.tensor_scalar(
                                            pt[:].bitcast(mybir.dt.uint16),
                                            sps[:], AEXP, BEXP,
                                            OP.mult, OP.add)
                                    else:
                                        nc.scalar.activation(
                                            pt[:], sps[:], AF.Exp, scale=SCALE)
                                    pts.append(pt)
                                elif variant == "pvonly":
                                    pts.append(pt_fix[hloc % 2])
                            if do_pv:
                                if prev is not None:
                                    emit_pv(prev)
                                prev = (kt, pts)
                        if do_pv:
                            emit_pv(prev)

                        # normalize into attnT (A rows 0:64, B rows 64:128)
                        if do_epi:
                          for row0, outp in [(0, outA), (HD, outB)]:
                            rep = rep_pool.tile([HD, QHW], FP32, tag="rep")
                            nc.vector.reciprocal(
                                rep[0:1, :], outp[HD:HD + 1, :])
                            nc.gpsimd.partition_broadcast(
                                rep[:], rep[0:1, :])
                            nc.vector.tensor_tensor(
                                attnT[row0:row0 + HD, p, q0:q0 + QHW],
                                outp[0:HD, :], rep[:], OP.mult)

                # ---- phase 3: out projection -----------------------------
                if variant not in att_variants:
                  with (
                      tc.tile_pool(name="opsum", bufs=4,
                                   space="PSUM") as opsum,
                      tc.tile_pool(name="ystage_pool", bufs=4) as ystage_pool,
                  ):
                    for co in range(CT):
                      for tb in range(NTB):
                        ps = opsum.tile([P, TBW], FP32, tag="op")
                        for ci in range(NPACK):
                            nc.tensor.matmul(
                                ps[:],
                                woTs[:, ci, co * P:(co + 1) * P],
                                attnT[:, ci, tb * TBW:(tb + 1) * TBW],
                                start=(ci == 0), stop=(ci == NPACK - 1))
                        yst = ystage_pool.tile([P, TBW], FP32, tag="yst")
                        nc.vector.tensor_scalar_add(
                            yst[:], ps[:], bos[:, co:co + 1])
                        nc.sync.dma_start(
                            yT_d[co * P:(co + 1) * P, tb * TBW:(tb + 1) * TBW],
                            yst[:])
    nc.compile()
    return nc


def _to_bf16(a):
    import ml_dtypes
    return np.asarray(a, np.float32).astype(ml_dtypes.bfloat16)


def _prep_core_inputs(x, Wqkv, bqkv, Wo, bo, core):
    b, g = core // 2, core % 2
    f32 = np.float32

    xT = _to_bf16(np.ascontiguousarray(x[b].T))

    wqkT = np.empty((D, NPACK, 256), f32)
    bqk = np.empty((P, NPACK, 2), f32)
    for p in range(NPACK):
        rows_q, rows_k = [], []
        for j in range(2):
            h = 8 * g + 2 * p + j
            rows_q.append(slice(192 * h, 192 * h + 64))
            rows_k.append(slice(192 * h + 64, 192 * h + 128))
        Q2 = np.vstack([Wqkv[rows_q[0]], Wqkv[rows_q[1]]])   # [128, D]
        K2 = np.vstack([Wqkv[rows_k[0]], Wqkv[rows_k[1]]])
        wqkT[:, p, :128] = Q2.T
        wqkT[:, p, 128:] = K2.T
        bqk[:, p, 0] = np.concatenate([bqkv[rows_q[0]], bqkv[rows_q[1]]])
        bqk[:, p, 1] = np.concatenate([bqkv[rows_k[0]], bqkv[rows_k[1]]])

    rows_v = [slice(192 * (8 * g + h) + 128, 192 * (8 * g + h) + 192)
              for h in range(HPC)]
    Wv = np.vstack([Wqkv[r] for r in rows_v])                # [512, D]
    wvT = _to_bf16(np.ascontiguousarray(Wv.T))

    woT = np.ascontiguousarray(Wo[:, 512 * g:512 * (g + 1)].T)  # [512, D]

    # fold V-bias through the out-projection: bo' = [g==0]*bo + woT.T @ bv
    bv_flat = np.empty(512, f32)
    for p in range(NPACK):
        bv_flat[128 * p:128 * p + 64] = bqkv[rows_v[2 * p]]
        bv_flat[128 * p + 64:128 * (p + 1)] = bqkv[rows_v[2 * p + 1]]
    bo_eff = (bo.astype(f32) if g == 0 else np.zeros(D, f32)) \
        + woT.T.astype(f32) @ bv_flat
    bo2 = np.ascontiguousarray(bo_eff.reshape(CT, P).T)

    return {
        "xT": xT, "wqkT": _to_bf16(wqkT), "bqk": bqk, "wvT": wvT,
        "woT": _to_bf16(woT), "bo": bo2,
    }


_NC_CACHE = {}


def kernel(x, Wqkv, bqkv, Wo, bo, _reps: int = 1,
           _return_raw: bool = False):
    x = np.asarray(x, np.float32)
    Wqkv = np.asarray(Wqkv, np.float32)
    bqkv = np.asarray(bqkv, np.float32)
    Wo = np.asarray(Wo, np.float32)
    bo = np.asarray(bo, np.float32)

    in_maps = [_prep_core_inputs(x, Wqkv, bqkv, Wo, bo, c)
               for c in range(N_CORES)]

    if _reps not in _NC_CACHE:
        _NC_CACHE[_reps] = build_nc(_reps)
    nc = _NC_CACHE[_reps]

    res = run_bass_kernel_spmd(nc, in_maps, core_ids=list(range(N_CORES)))
    if _return_raw:
        return res

    y = np.empty((B, T, D), np.float32)
    for b in range(B):
        yt = res.results[2 * b]["yT"] + res.results[2 * b + 1]["yT"]
        y[b] = yt.T
    return y

